# revision 4
# baseline (speedup 1.0000x reference)
"""MoE grouped-FFN kernel for Trainium2 (8 NeuronCores, expert-parallel).

Problem: x [1, 2048, 1024] fp32, 32 experts x 64 tokens each,
per-expert FFN 1024 -> 4096 (gelu) -> 1024.

Sharding: expert-parallel, 4 experts per core. Tokens are statically
pre-chunked per expert (dim 1 == E*C), so each core just gets its 4
experts' token rows + weights; outputs concatenate back. No collectives.

Per-core dataflow (all matmuls in float32r = full-rate fp32 PE mode):
  mm1: h[c,512f] += xT[k].T @ W1[k, fchunk]   (tokens on M=64, K-accum in PSUM)
  PE-transpose h -> hT [f, c],  ACT: hT_sb = gelu(hT + b1) (per-partition bias)
  mm2: out[c,512d] += hT[ft].T @ W2[ft, dchunk]
  DVE: out_sb = out_psum + b2  (evacuation fused with bias add)
"""

import os
import numpy as np

E, C, D, F = 32, 64, 1024, 4096
N_CORES = 8
E_LOC = E // N_CORES  # experts per core
P = 128
FCHUNK = 512  # matmul moving-operand max for 4-byte dtypes
KT1 = D // P  # 8 K-tiles in mm1
FT = F // P  # 32 f-tiles (contraction tiles for mm2)

_CACHE = {}
LAST_RESULTS = None  # BassKernelResults of the most recent run (for profiling)


def _build_program(act="gelu"):
    import concourse.bacc as bacc
    import concourse.tile as tile
    import concourse.mybir as mybir

    f32 = mybir.dt.float32
    f32r = mybir.dt.float32r
    # CoreSim doesn't implement the Gelu LUTs; "tanh" is a sim-only stand-in
    # used by test.py to validate everything except the activation itself.
    GELU = {
        "gelu": mybir.ActivationFunctionType.Gelu_apprx_tanh,
        "tanh": mybir.ActivationFunctionType.Tanh,
    }[act]
    ADD = mybir.AluOpType.add

    nc = bacc.Bacc("TRN2", target_bir_lowering=False, debug=False)

    xT_d = nc.declare_dram_parameter("xT", [P, E_LOC, KT1, C], f32r, isOutput=False)
    w1_d = nc.declare_dram_parameter("w1", [E_LOC, D, F], f32r, isOutput=False)
    w2_d = nc.declare_dram_parameter("w2", [E_LOC, F, D], f32r, isOutput=False)
    b1_d = nc.declare_dram_parameter("b1t", [P, E_LOC, FT], f32, isOutput=False)
    b2_d = nc.declare_dram_parameter("b2r", [C, E_LOC, D], f32, isOutput=False)
    id_d = nc.declare_dram_parameter("ident", [C, C], f32, isOutput=False)
    out_d = nc.declare_dram_parameter("out", [E_LOC * C, D], f32, isOutput=True)

    # DRAM views with the partition dim first for clean DMA descriptors.
    w1_ap = w1_d.ap().rearrange("e (k p) f -> p e k f", p=P)  # [128, 4, 8, 4096]
    w2_ap = w2_d.ap().rearrange("e (a p) d -> p e a d", p=P)  # [128, 4, 32, 1024]

    with tile.TileContext(nc) as tc:
        with (
            tc.tile_pool(name="const", bufs=1) as const_pool,
            tc.tile_pool(name="w1", bufs=4) as w1_pool,
            tc.tile_pool(name="w2", bufs=3) as w2_pool,
            tc.tile_pool(name="hs", bufs=8) as hs_pool,
            tc.tile_pool(name="ht", bufs=2) as ht_pool,
            tc.tile_pool(name="os", bufs=2) as os_pool,
            tc.tile_pool(name="ph", bufs=2, space="PSUM") as ph_pool,
            tc.tile_pool(name="pt", bufs=4, space="PSUM") as pt_pool,
            tc.tile_pool(name="po", bufs=2, space="PSUM") as po_pool,
        ):
            xT_sb = const_pool.tile([P, E_LOC, KT1, C], f32r, tag="xt")
            nc.sync.dma_start(out=xT_sb, in_=xT_d.ap())
            b1_sb = const_pool.tile([P, E_LOC, FT], f32, tag="b1")
            nc.sync.dma_start(out=b1_sb, in_=b1_d.ap())
            b2_sb = const_pool.tile([C, E_LOC, D], f32, tag="b2")
            nc.sync.dma_start(out=b2_sb, in_=b2_d.ap())
            id_sb = const_pool.tile([C, C], f32, tag="id")
            nc.sync.dma_start(out=id_sb, in_=id_d.ap())

            for e in range(E_LOC):
                # ---- phase 1: h = x_e @ W1_e, keep pre-activation in SBUF
                hs_tiles = []
                for c in range(F // FCHUNK):
                    w1t = w1_pool.tile([P, KT1, FCHUNK], f32r, tag="w1t")
                    nc.sync.dma_start(
                        out=w1t,
                        in_=w1_ap[:, e, :, c * FCHUNK : (c + 1) * FCHUNK],
                    )
                    hp = ph_pool.tile([C, FCHUNK], f32, tag="hp")
                    for k in range(KT1):
                        nc.tensor.matmul(
                            hp,
                            lhsT=xT_sb[:, e, k, :],
                            rhs=w1t[:, k, :],
                            start=(k == 0),
                            stop=(k == KT1 - 1),
                        )
                    hs = hs_pool.tile([C, FCHUNK], f32, tag="hs")
                    nc.vector.tensor_copy(out=hs, in_=hp)
                    hs_tiles.append(hs)

                # ---- transpose h -> hT and apply bias + gelu per f-tile
                hT = ht_pool.tile([P, FT, C], f32r, tag="ht")
                for c in range(F // FCHUNK):
                    for t in range(FCHUNK // P):
                        ft = (FCHUNK // P) * c + t
                        tp = pt_pool.tile([P, C], f32, tag="tp")
                        nc.tensor.transpose(
                            tp,
                            in_=hs_tiles[c][:, t * P : (t + 1) * P],
                            identity=id_sb,
                        )
                        nc.scalar.activation(
                            out=hT[:, ft, :],
                            in_=tp,
                            func=GELU,
                            bias=b1_sb[:, e, ft : ft + 1],
                        )

                # ---- phase 2: out_e = gelu(h) @ W2_e + b2_e
                op0 = po_pool.tile([C, FCHUNK], f32, tag="op")
                op1 = po_pool.tile([C, FCHUNK], f32, tag="op")
                for fb in range(FT // 4):
                    w2t = w2_pool.tile([P, 4, D], f32r, tag="w2t")
                    nc.scalar.dma_start(
                        out=w2t, in_=w2_ap[:, e, 4 * fb : 4 * fb + 4, :]
                    )
                    for j in range(4):
                        ft = 4 * fb + j
                        nc.tensor.matmul(
                            op0,
                            lhsT=hT[:, ft, :],
                            rhs=w2t[:, j, 0:FCHUNK],
                            start=(ft == 0),
                            stop=(ft == FT - 1),
                        )
                        nc.tensor.matmul(
                            op1,
                            lhsT=hT[:, ft, :],
                            rhs=w2t[:, j, FCHUNK:D],
                            start=(ft == 0),
                            stop=(ft == FT - 1),
                        )
                os_t = os_pool.tile([C, D], f32, tag="os")
                nc.vector.tensor_tensor(
                    os_t[:, 0:FCHUNK], op0, b2_sb[:, e, 0:FCHUNK], ADD
                )
                nc.vector.tensor_tensor(
                    os_t[:, FCHUNK:D], op1, b2_sb[:, e, FCHUNK:D], ADD
                )
                nc.sync.dma_start(
                    out=out_d.ap()[e * C : (e + 1) * C, :], in_=os_t
                )

    nc.compile()
    return nc


def _get_program(act="gelu"):
    if act not in _CACHE:
        _CACHE[act] = _build_program(act)
    return _CACHE[act]


def make_in_maps(x, W1, b1, W2, b2):
    x = np.ascontiguousarray(np.asarray(x, dtype=np.float32))
    W1 = np.ascontiguousarray(np.asarray(W1, dtype=np.float32))
    b1 = np.ascontiguousarray(np.asarray(b1, dtype=np.float32))
    W2 = np.ascontiguousarray(np.asarray(W2, dtype=np.float32))
    b2 = np.ascontiguousarray(np.asarray(b2, dtype=np.float32))
    ident = np.eye(C, dtype=np.float32)
    in_maps = []
    for i in range(N_CORES):
        lo, hi = i * E_LOC, (i + 1) * E_LOC
        xc = x[0, lo * C : hi * C, :].reshape(E_LOC, C, KT1, P)
        xT = np.ascontiguousarray(xc.transpose(3, 0, 2, 1))  # [128, e, k, c]
        b1t = np.ascontiguousarray(
            b1[lo:hi].reshape(E_LOC, FT, P).transpose(2, 0, 1)
        )  # [128, e, ft]
        b2r = np.ascontiguousarray(
            np.broadcast_to(b2[lo:hi][None], (C, E_LOC, D))
        )  # [64, e, d]
        in_maps.append(
            {
                "xT": xT,
                "w1": np.ascontiguousarray(W1[lo:hi]),
                "w2": np.ascontiguousarray(W2[lo:hi]),
                "b1t": b1t,
                "b2r": b2r,
                "ident": ident,
            }
        )
    return in_maps


def kernel(x, W1, b1, W2, b2):
    global LAST_RESULTS
    from concourse.bass_utils import run_bass_kernel_spmd

    nc = _get_program()
    in_maps = make_in_maps(x, W1, b1, W2, b2)
    trace = bool(int(os.environ.get("KERNEL_TRACE", "0")))
    res = run_bass_kernel_spmd(nc, in_maps, list(range(N_CORES)), trace=trace)
    LAST_RESULTS = res
    out = np.concatenate([r["out"] for r in res.results], axis=0)
    return out.reshape(1, E * C, D).astype(np.float32)


# revision 7
# speedup vs baseline: 76.6958x; 76.6958x over previous
"""MoE grouped-FFN kernel for Trainium2 (8 NeuronCores, expert-parallel).

Problem: x [1, 2048, 1024] fp32, 32 experts x 64 tokens each,
per-expert FFN 1024 -> 4096 (gelu) -> 1024.

Sharding: expert-parallel, 4 experts per core. Tokens are statically
pre-chunked per expert (dim 1 == E*C), so each core just gets its 4
experts' token rows + weights; outputs concatenate back. No collectives.

Per-core dataflow (all matmuls in float32r = full-rate fp32 PE mode):
  mm1: h[c,512f] += xT[k].T @ W1[k, fchunk]   (tokens on M=64, K-accum in PSUM)
  PE-transpose h -> hT [f, c],  ACT: hT_sb = gelu(hT + b1) (per-partition bias)
  mm2: out[c,512d] += hT[ft].T @ W2[ft, dchunk]
  DVE: out_sb = out_psum + b2  (evacuation fused with bias add)
"""

import os
import numpy as np

E, C, D, F = 32, 64, 1024, 4096
N_CORES = 8
E_LOC = E // N_CORES  # experts per core
P = 128
FCHUNK = 512  # matmul moving-operand max for 4-byte dtypes
KT1 = D // P  # 8 K-tiles in mm1
FT = F // P  # 32 f-tiles (contraction tiles for mm2)

_CACHE = {}
LAST_RESULTS = None  # BassKernelResults of the most recent run (for profiling)


def _build_program(act="gelu", repeats=1):
    import contextlib

    import concourse.bacc as bacc
    import concourse.tile as tile
    import concourse.mybir as mybir

    f32 = mybir.dt.float32
    f32r = mybir.dt.float32r
    # CoreSim doesn't implement the Gelu LUTs; "tanh" is a sim-only stand-in
    # used by test.py to validate everything except the activation itself.
    GELU = {
        "gelu": mybir.ActivationFunctionType.Gelu_apprx_tanh,
        "tanh": mybir.ActivationFunctionType.Tanh,
    }[act]
    ADD = mybir.AluOpType.add

    nc = bacc.Bacc("TRN2", target_bir_lowering=False, debug=False)

    xT_d = nc.declare_dram_parameter("xT", [P, E_LOC, KT1, C], f32r, isOutput=False)
    w1_d = nc.declare_dram_parameter("w1", [E_LOC, D, F], f32r, isOutput=False)
    w2_d = nc.declare_dram_parameter("w2", [E_LOC, F, D], f32r, isOutput=False)
    b1_d = nc.declare_dram_parameter("b1t", [P, E_LOC, FT], f32, isOutput=False)
    b2_d = nc.declare_dram_parameter("b2r", [C, E_LOC, D], f32, isOutput=False)
    id_d = nc.declare_dram_parameter("ident", [C, C], f32, isOutput=False)
    out_d = nc.declare_dram_parameter("out", [E_LOC * C, D], f32, isOutput=True)

    # DRAM views with the partition dim first for clean DMA descriptors.
    w1_ap = w1_d.ap().rearrange("e (k p) f -> p e k f", p=P)  # [128, 4, 8, 4096]
    w2_ap = w2_d.ap().rearrange("e (a p) d -> p e a d", p=P)  # [128, 4, 32, 1024]

    with tile.TileContext(nc) as tc:
        with (
            tc.tile_pool(name="const", bufs=1) as const_pool,
            tc.tile_pool(name="w1", bufs=4) as w1_pool,
            tc.tile_pool(name="w2", bufs=3) as w2_pool,
            tc.tile_pool(name="hs", bufs=8) as hs_pool,
            tc.tile_pool(name="ht", bufs=2) as ht_pool,
            tc.tile_pool(name="os", bufs=2) as os_pool,
            tc.tile_pool(name="ph", bufs=2, space="PSUM") as ph_pool,
            tc.tile_pool(name="pt", bufs=4, space="PSUM") as pt_pool,
            tc.tile_pool(name="po", bufs=2, space="PSUM") as po_pool,
        ):
            xT_sb = const_pool.tile([P, E_LOC, KT1, C], f32r, tag="xt")
            nc.sync.dma_start(out=xT_sb, in_=xT_d.ap())
            b1_sb = const_pool.tile([P, E_LOC, FT], f32, tag="b1")
            nc.sync.dma_start(out=b1_sb, in_=b1_d.ap())
            b2_sb = const_pool.tile([C, E_LOC, D], f32, tag="b2")
            nc.sync.dma_start(out=b2_sb, in_=b2_d.ap())
            id_sb = const_pool.tile([C, C], f32, tag="id")
            nc.sync.dma_start(out=id_sb, in_=id_d.ap())

            # repeats>1 wraps the whole computation in a hardware loop so a
            # single execute measures R back-to-back runs (benchmarking only).
            rep_ctx = (
                tc.For_i(0, repeats, 1) if repeats > 1 else contextlib.nullcontext()
            )
            with rep_ctx:
                _emit_body(
                    nc, tc, GELU, ADD,
                    xT_sb, b1_sb, b2_sb, id_sb,
                    w1_ap, w2_ap, out_d,
                    w1_pool, w2_pool, hs_pool, ht_pool, os_pool,
                    ph_pool, pt_pool, po_pool,
                    f32, f32r,
                )

    nc.compile()
    return nc


def _emit_body(
    nc, tc, GELU, ADD,
    xT_sb, b1_sb, b2_sb, id_sb,
    w1_ap, w2_ap, out_d,
    w1_pool, w2_pool, hs_pool, ht_pool, os_pool,
    ph_pool, pt_pool, po_pool,
    f32, f32r,
):
    if True:
        if True:
            for e in range(E_LOC):
                # ---- phase 1: h = x_e @ W1_e, keep pre-activation in SBUF
                hs_tiles = []
                for c in range(F // FCHUNK):
                    w1t = w1_pool.tile([P, KT1, FCHUNK], f32r, tag="w1t")
                    nc.sync.dma_start(
                        out=w1t,
                        in_=w1_ap[:, e, :, c * FCHUNK : (c + 1) * FCHUNK],
                    )
                    hp = ph_pool.tile([C, FCHUNK], f32, tag="hp")
                    for k in range(KT1):
                        nc.tensor.matmul(
                            hp,
                            lhsT=xT_sb[:, e, k, :],
                            rhs=w1t[:, k, :],
                            start=(k == 0),
                            stop=(k == KT1 - 1),
                        )
                    hs = hs_pool.tile([C, FCHUNK], f32, tag="hs")
                    nc.vector.tensor_copy(out=hs, in_=hp)
                    hs_tiles.append(hs)

                # ---- transpose h -> hT and apply bias + gelu per f-tile
                hT = ht_pool.tile([P, FT, C], f32r, tag="ht")
                for c in range(F // FCHUNK):
                    for t in range(FCHUNK // P):
                        ft = (FCHUNK // P) * c + t
                        tp = pt_pool.tile([P, C], f32, tag="tp")
                        nc.tensor.transpose(
                            tp,
                            in_=hs_tiles[c][:, t * P : (t + 1) * P],
                            identity=id_sb,
                        )
                        nc.scalar.activation(
                            out=hT[:, ft, :],
                            in_=tp,
                            func=GELU,
                            bias=b1_sb[:, e, ft : ft + 1],
                        )

                # ---- phase 2: out_e = gelu(h) @ W2_e + b2_e
                op0 = po_pool.tile([C, FCHUNK], f32, tag="op")
                op1 = po_pool.tile([C, FCHUNK], f32, tag="op")
                for fb in range(FT // 4):
                    w2t = w2_pool.tile([P, 4, D], f32r, tag="w2t")
                    nc.scalar.dma_start(
                        out=w2t, in_=w2_ap[:, e, 4 * fb : 4 * fb + 4, :]
                    )
                    for j in range(4):
                        ft = 4 * fb + j
                        nc.tensor.matmul(
                            op0,
                            lhsT=hT[:, ft, :],
                            rhs=w2t[:, j, 0:FCHUNK],
                            start=(ft == 0),
                            stop=(ft == FT - 1),
                        )
                        nc.tensor.matmul(
                            op1,
                            lhsT=hT[:, ft, :],
                            rhs=w2t[:, j, FCHUNK:D],
                            start=(ft == 0),
                            stop=(ft == FT - 1),
                        )
                os_t = os_pool.tile([C, D], f32, tag="os")
                nc.vector.tensor_tensor(
                    os_t[:, 0:FCHUNK], op0, b2_sb[:, e, 0:FCHUNK], ADD
                )
                nc.vector.tensor_tensor(
                    os_t[:, FCHUNK:D], op1, b2_sb[:, e, FCHUNK:D], ADD
                )
                nc.sync.dma_start(
                    out=out_d.ap()[e * C : (e + 1) * C, :], in_=os_t
                )


def _get_program(act="gelu", repeats=1):
    key = (act, repeats)
    if key not in _CACHE:
        _CACHE[key] = _build_program(act, repeats)
    return _CACHE[key]


def make_in_maps(x, W1, b1, W2, b2):
    x = np.ascontiguousarray(np.asarray(x, dtype=np.float32))
    W1 = np.ascontiguousarray(np.asarray(W1, dtype=np.float32))
    b1 = np.ascontiguousarray(np.asarray(b1, dtype=np.float32))
    W2 = np.ascontiguousarray(np.asarray(W2, dtype=np.float32))
    b2 = np.ascontiguousarray(np.asarray(b2, dtype=np.float32))
    ident = np.eye(C, dtype=np.float32)
    in_maps = []
    for i in range(N_CORES):
        lo, hi = i * E_LOC, (i + 1) * E_LOC
        xc = x[0, lo * C : hi * C, :].reshape(E_LOC, C, KT1, P)
        xT = np.ascontiguousarray(xc.transpose(3, 0, 2, 1))  # [128, e, k, c]
        b1t = np.ascontiguousarray(
            b1[lo:hi].reshape(E_LOC, FT, P).transpose(2, 0, 1)
        )  # [128, e, ft]
        b2r = np.ascontiguousarray(
            np.broadcast_to(b2[lo:hi][None], (C, E_LOC, D))
        )  # [64, e, d]
        in_maps.append(
            {
                "xT": xT,
                "w1": np.ascontiguousarray(W1[lo:hi]),
                "w2": np.ascontiguousarray(W2[lo:hi]),
                "b1t": b1t,
                "b2r": b2r,
                "ident": ident,
            }
        )
    return in_maps


def kernel(x, W1, b1, W2, b2):
    global LAST_RESULTS
    from concourse.bass_utils import run_bass_kernel_spmd

    nc = _get_program()
    in_maps = make_in_maps(x, W1, b1, W2, b2)
    trace = bool(int(os.environ.get("KERNEL_TRACE", "0")))
    res = run_bass_kernel_spmd(nc, in_maps, list(range(N_CORES)), trace=trace)
    LAST_RESULTS = res
    out = np.concatenate([r["out"] for r in res.results], axis=0)
    return out.reshape(1, E * C, D).astype(np.float32)
